# revision 12
# baseline (speedup 1.0000x reference)
"""GriffinLimVocoder Trainium2 kernel.

Pipeline per batch element (one NeuronCore each, batch=8, cores=8):
  1. InverseMelScale: 50 SGD iterations on spec (freq=401 x T=512).
  2. Griffin-Lim: 30 iterations of ISTFT -> STFT (800-pt DFTs as matmuls)
     with overlap-add / re-framing done in a transposed (sample-major) layout.
  3. Final ISTFT + per-sample peak normalization.

All DFTs are dense matmuls against precomputed cos/sin matrices (window and
bulk 1/wsq = 1/1.5 folded in).  Layouts keep frames on the free dimension so
overlap-add and re-framing become free-dim shifted adds / slices; the only
cross-partition work (reflect-pad edge columns) is done with tiny constant
permutation matmuls.
"""

import os
from contextlib import ExitStack

import numpy as np

SR = 8000
N_FFT = 800
HOP = 200
N_MELS = 80
N_FREQ = 401
T = 512                      # frames
STFT_ITER = 50
GL_ITER = 30
SGD_LR = 0.1
SGD_MOM = 0.9
GL_MOM = 0.99
BETA = GL_MOM / (1.0 + GL_MOM)
TARGET_DB = -0.1
TARGET_AMP = 10.0 ** (TARGET_DB / 20.0)
B = 8
NCORES = 8

YLEN = N_FFT + HOP * (T - 1)          # 103000
NB = YLEN // HOP                      # 515 blocks of 200
OUT_LEN = YLEN - N_FFT                # 102200
NQ = OUT_LEN // HOP                   # 511 output blocks

FC = [(0, 128), (128, 128), (256, 128), (384, 17)]   # freq chunks
SCALE = 2.0 / (B * N_MELS)

EPS_GL = 1e-16


# ----------------------------------------------------------------------------
# constants
# ----------------------------------------------------------------------------

def _hann(n):
    return 0.5 - 0.5 * np.cos(2.0 * np.pi * np.arange(n) / n)


def _mel_fb():
    all_freqs = np.linspace(0.0, SR / 2, N_FREQ)
    hz2mel = lambda f: 2595.0 * np.log10(1.0 + f / 700.0)
    mel2hz = lambda m: 700.0 * (10.0 ** (m / 2595.0) - 1.0)
    f_pts = mel2hz(np.linspace(hz2mel(0.0), hz2mel(SR / 2), N_MELS + 2))
    f_diff = f_pts[1:] - f_pts[:-1]
    slopes = f_pts[None, :] - all_freqs[:, None]
    down = -slopes[:, :-2] / f_diff[:-1]
    up = slopes[:, 2:] / f_diff[1:]
    return np.maximum(0.0, np.minimum(down, up))   # (freq, n_mels)


def _wsq():
    w2 = _hann(N_FFT) ** 2
    wsq = np.zeros(YLEN)
    for t in range(T):
        wsq[t * HOP: t * HOP + N_FFT] += w2
    return wsq


def _edge_spec(wsq):
    """Edge (reflect-pad) columns of the re-framing input, as tiny matmuls.

    Target col values (already including the 1.5/wsq correction relative to
    the bulk scaling folded into the forward-DFT weights):
      col 0:   y[800-u]                    u in [0,200)
      col 1:   1.5*y[600-u]/wsq[600-u]
      col 513: 1.5*y[102598-u]/wsq[102598-u]
      col 514: y[102398-u]
    Returns (entries, mats): entries = list of (tcol, ho, [(hs, blk, mi)]),
    mats = (100, NM, 100) with mats[s, mi, ul]: out[ul] += mat[s]*YT_hs[s, blk].
    """
    defs = {
        0: (lambda u: 800 - u, lambda u: 1.0),
        1: (lambda u: 600 - u, lambda u: 1.5 / wsq[600 - u]),
        513: (lambda u: 102598 - u, lambda u: 1.5 / wsq[102598 - u]),
        514: (lambda u: 102398 - u, lambda u: 1.0),
    }
    entries = []
    mats = []
    for tcol, (idxf, wf) in defs.items():
        for ho in (0, 1):
            buckets = {}
            for ul in range(100):
                u = 100 * ho + ul
                idx = idxf(u)
                blk, sl = idx // HOP, idx % HOP
                hs, r = sl // 100, sl % 100
                m = buckets.setdefault((hs, blk), np.zeros((100, 100)))
                m[r, ul] += wf(u)
            srcs = []
            for (hs, blk), m in sorted(buckets.items()):
                srcs.append((hs, blk, len(mats)))
                mats.append(m)
            entries.append((tcol, ho, srcs))
    return entries, np.stack(mats, axis=1)   # (100, NM, 100)


def build_consts(dtype=np.float32):
    j = np.arange(N_FFT)
    f = np.arange(N_FREQ)
    ang = 2.0 * np.pi * np.outer(f, j) / N_FFT        # (freq, nfft)
    C, S = np.cos(ang), np.sin(ang)
    w = _hann(N_FFT)
    cf = np.full(N_FREQ, 2.0)
    cf[0] = cf[N_FREQ - 1] = 1.0
    # irfft (+window):  frames_w = CRw.T @ Re + CIw.T @ Im  -> (800, T)
    CRw = (cf[:, None] / N_FFT) * C * w[None, :]       # (401, 800)
    CIw = (-cf[:, None] / N_FFT) * S * w[None, :]
    # rfft (+window, + bulk 1/1.5):  Re = CRT.T @ F2, Im = CIT.T @ F2
    CRT = (w[None, :] * C / 1.5).T                     # (800, 401)
    CIT = (-w[None, :] * S / 1.5).T

    fb = _mel_fb()                                     # (401, 80)
    Wg = (-SCALE) * fb.T                               # (80, 401)

    wsq = _wsq()
    entries, emats = _edge_spec(wsq)
    e2 = 1.5 / wsq[400:600]
    e512 = 1.5 / wsq[102400:102600]
    iw0 = 1.0 / np.maximum(wsq[400:600], 1e-11)
    iw510 = 1.0 / np.maximum(wsq[102400:102600], 1e-11)
    ecols = np.stack([e2[:100], e2[100:], e512[:100], e512[100:]], 1)     # (100,4)
    iwcols = np.stack([iw0[:100], iw0[100:], iw510[:100], iw510[100:]], 1)

    def pad_chunks(M, inner, tagshape):
        # M (401, inner) -> (128, 4, inner) zero-padded freq chunks
        out = np.zeros((128, 4, inner))
        for c, (f0, sz) in enumerate(FC):
            out[:sz, c, :] = M[f0:f0 + sz, :]
        return out

    consts = {
        "Wa_r": pad_chunks(CRw, N_FFT, None),              # (128,4,800)
        "Wa_i": pad_chunks(CIw, N_FFT, None),
        "Wc_r": CRT.reshape(4, 200, N_FREQ).transpose(1, 0, 2).reshape(
            200, 4, N_FREQ).swapaxes(0, 0),                # placeholder fixed below
        "fb": pad_chunks(fb, N_MELS, None),                # (128,4,80)
        "Wg": Wg,                                          # (80,401)
        "emats": emats,                                    # (100, NM, 100)
        "ecols": ecols,
        "iwcols": iwcols,
        "negI": -BETA * np.eye(128),
        "ones1": TARGET_AMP * np.ones((1, 128)),
    }
    # Wc layout: (100, 8, 401): chunk k = rows 100k..100k+99 of the 800-row mat
    consts["Wc_r"] = CRT.reshape(8, 100, N_FREQ).transpose(1, 0, 2)
    consts["Wc_i"] = CIT.reshape(8, 100, N_FREQ).transpose(1, 0, 2)
    consts = {k: np.ascontiguousarray(v, dtype=dtype) for k, v in consts.items()}
    consts["_entries"] = entries
    return consts


def host_inits():
    """Reproduce the reference's jax PRNG initializations on CPU."""
    import jax
    cpu = jax.devices("cpu")[0]
    with jax.default_device(cpu):
        k1, k2 = jax.random.split(jax.random.key(1))
        spec0 = jax.random.uniform(k1, (B, T, N_FREQ), dtype=np.float32)
        kr, ki = jax.random.split(k2)
        ar = jax.random.uniform(kr, (B, N_FREQ, T), dtype=np.float32)
        ai = jax.random.uniform(ki, (B, N_FREQ, T), dtype=np.float32)
        return (np.asarray(spec0).transpose(0, 2, 1).copy(),   # (B,401,512)
                np.asarray(ar), np.asarray(ai))


# ----------------------------------------------------------------------------
# numpy prototype (validation/debug path; mirrors the device algorithm)
# ----------------------------------------------------------------------------

def proto(x, gl_iters=GL_ITER, sgd_iters=STFT_ITER, dtype=np.float64):
    c = build_consts(dtype=dtype)
    entries = c["_entries"]
    spec0T, ar, ai = host_inits()
    out = np.zeros((B, 1, OUT_LEN), dtype=dtype)

    def unchunk(M, inner):   # (128,4,inner) -> (401, inner)
        return np.concatenate([M[:sz, i, :] for i, (f0, sz) in enumerate(FC)], 0)

    CRw = unchunk(c["Wa_r"], N_FFT)
    CIw = unchunk(c["Wa_i"], N_FFT)
    CRT = c["Wc_r"].transpose(1, 0, 2).reshape(800, N_FREQ)
    CIT = c["Wc_i"].transpose(1, 0, 2).reshape(800, N_FREQ)
    fb = unchunk(c["fb"], N_MELS)
    Wg, emats, ecols, iwcols = c["Wg"], c["emats"], c["ecols"], c["iwcols"]

    def ola(scr, sci):
        fw = CRw.T @ scr + CIw.T @ sci                   # (800, 512)
        Y = np.zeros((200, NB), dtype)
        for jj in range(4):
            Y[:, jj:jj + T] += fw[200 * jj:200 * jj + 200, :]
        return Y

    for b in range(B):
        mel = x[b].astype(dtype)                         # (80,512)
        spec = spec0T[b].astype(dtype)                   # (401,512)
        vel = np.zeros_like(spec)
        for _ in range(sgd_iters):
            diff = mel - fb.T @ spec
            g = Wg.T @ diff
            vel = SGD_MOM * vel + g
            spec = np.maximum(spec - SGD_LR * vel, 0.0)
        mag = np.sqrt(spec)
        scr, sci = mag * ar[b], mag * ai[b]
        tpr = tpi = np.zeros_like(scr)
        for _ in range(gl_iters):
            Y = ola(scr, sci)
            X = Y.copy()
            for tcol, ho, srcs in entries:
                acc = np.zeros(100, dtype)
                for hs, blk, mi in srcs:
                    Yh = Y[hs * 100:hs * 100 + 100, :]
                    acc += emats[:, mi, :].T @ Yh[:, blk]
                X[ho * 100:ho * 100 + 100, tcol] = acc
            X[:100, 2] = Y[:100, 2] * ecols[:, 0]
            X[100:, 2] = Y[100:, 2] * ecols[:, 1]
            X[:100, 512] = Y[:100, 512] * ecols[:, 2]
            X[100:, 512] = Y[100:, 512] * ecols[:, 3]
            F2 = np.zeros((800, T), dtype)
            for k in range(8):
                hs, cc = k % 2, k // 2
                F2[100 * k:100 * k + 100, :] = X[hs * 100:hs * 100 + 100, cc:cc + T]
            rebr = CRT.T @ F2
            rebi = CIT.T @ F2
            nr, ni = rebr - BETA * tpr, rebi - BETA * tpi
            tpr, tpi = rebr, rebi
            d = 1.0 / (np.sqrt(nr * nr + ni * ni) + EPS_GL)
            e = mag * d
            scr, sci = nr * e, ni * e
        Y = ola(scr, sci)
        W = Y[:, 2:2 + NQ] / 1.5
        W[:100, 0] = Y[:100, 2] * iwcols[:, 0]
        W[100:, 0] = Y[100:, 2] * iwcols[:, 1]
        W[:100, NQ - 1] = Y[:100, 512] * iwcols[:, 2]
        W[100:, NQ - 1] = Y[100:, 512] * iwcols[:, 3]
        wav = W.T.reshape(-1)
        out[b, 0, :] = wav * (TARGET_AMP / np.max(np.abs(wav)))
    return out.astype(np.float32)


# ----------------------------------------------------------------------------
# Bass kernel
# ----------------------------------------------------------------------------

_CACHE = {}


def build_bass(mm="float32r", sgd_iters=STFT_ITER, gl_iters=GL_ITER):
    import concourse.bass as bass
    import concourse.mybir as mybir
    import concourse.tile as tile
    from concourse import bacc
    from concourse.masks import make_identity

    f32 = mybir.dt.float32
    mmdt = getattr(mybir.dt, mm)
    ALU = mybir.AluOpType
    ACT = mybir.ActivationFunctionType

    consts = build_consts(np.float32)
    entries = consts.pop("_entries")
    NM = consts["emats"].shape[1]

    MMSET = {"spec0", "ang_r", "ang_i", "Wa_r", "Wa_i", "Wc_r", "Wc_i",
             "fb", "Wg", "emats", "negI"}
    nc = bacc.Bacc("TRN2", target_bir_lowering=False)
    dram = {}
    shapes = {
        "mel": (N_MELS, T), "spec0": (128, 4, T), "ang_r": (128, 4, T),
        "ang_i": (128, 4, T),
        "Wa_r": (128, 4, N_FFT), "Wa_i": (128, 4, N_FFT),
        "Wc_r": (100, 8, N_FREQ), "Wc_i": (100, 8, N_FREQ),
        "fb": (128, 4, N_MELS), "Wg": (N_MELS, N_FREQ),
        "emats": (100, NM, 100), "ecols": (100, 4), "iwcols": (100, 4),
        "negI": (128, 128), "ones1": (1, 128),
    }
    for name, shp in shapes.items():
        dt_ = mmdt if name in MMSET else f32
        dram[name] = nc.declare_dram_parameter(name, list(shp), dt_, isOutput=False)
    out_d = nc.declare_dram_parameter("out", [OUT_LEN], f32, isOutput=True)

    def mmul(out, lhsT, rhs, start, stop):
        nc.tensor.matmul(out, lhsT=lhsT, rhs=rhs, start=start, stop=stop)

    with ExitStack() as ctx:
        tc = ctx.enter_context(tile.TileContext(nc))
        const = ctx.enter_context(tc.tile_pool(name="const", bufs=1))
        state = ctx.enter_context(tc.tile_pool(name="state", bufs=1))
        work = ctx.enter_context(tc.tile_pool(name="work", bufs=3))
        ytp = ctx.enter_context(tc.tile_pool(name="ytp", bufs=2))
        ps = ctx.enter_context(tc.tile_pool(name="ps", bufs=8, space="PSUM"))

        # ---- load constants / inputs ------------------------------------
        sb = {}
        order = ["fb", "Wg", "mel", "spec0",                       # SGD first
                 "Wa_r", "Wa_i", "Wc_r", "Wc_i", "emats", "ecols",
                 "iwcols", "negI", "ones1", "ang_r", "ang_i"]
        for name in order:
            shp = shapes[name]
            pool = state if name in ("mel", "spec0", "ang_r", "ang_i") else const
            dt_ = mmdt if name in MMSET else f32
            sb[name] = pool.tile(list(shp), dt_, tag=name, name=name)
            nc.sync.dma_start(out=sb[name][...], in_=dram[name][...])
        ident = const.tile([128, 128], f32, tag="ident", name="ident")
        make_identity(nc, ident[...])

        spec, mel = sb["spec0"], sb["mel"]
        vel = state.tile([128, 4, T], f32, tag="vel", name="vel")
        nc.gpsimd.memset(vel[...], 0.0)

        # ---- SGD (InverseMelScale) --------------------------------------
        for it in range(sgd_iters):
            dp = ps.tile([128, T], f32, tag="ps", name="ps")
            for c, (f0, sz) in enumerate(FC):
                mmul(dp[:N_MELS], sb["fb"][:sz, c, :], spec[:sz, c, :],
                     start=(c == 0), stop=(c == 3))
            diff = work.tile([N_MELS, T], mmdt, tag="diff", name="diff")
            nc.vector.tensor_sub(diff[...], mel[...], dp[:N_MELS])
            for c, (f0, sz) in enumerate(FC):
                gp = ps.tile([128, T], f32, tag="ps", name="ps")
                mmul(gp[:sz], sb["Wg"][:, f0:f0 + sz], diff[...],
                     start=True, stop=True)
                nc.vector.scalar_tensor_tensor(
                    out=vel[:sz, c, :], in0=vel[:sz, c, :], scalar=SGD_MOM,
                    in1=gp[:sz], op0=ALU.mult, op1=ALU.add)
                stmp = work.tile([128, T], f32, tag="stmp", name="stmp")
                nc.vector.scalar_tensor_tensor(
                    out=stmp[:sz], in0=vel[:sz, c, :], scalar=-SGD_LR,
                    in1=spec[:sz, c, :], op0=ALU.mult, op1=ALU.add)
                nc.gpsimd.tensor_relu(out=spec[:sz, c, :], in_=stmp[:sz])

        # ---- mag, initial complex spec ----------------------------------
        mag = state.tile([128, 4, T], f32, tag="mag", name="mag")
        scr, sci = sb["ang_r"], sb["ang_i"]       # overwritten in place
        for c, (f0, sz) in enumerate(FC):
            nc.scalar.activation(out=mag[:sz, c, :], in_=spec[:sz, c, :],
                                 func=ACT.Sqrt)
            nc.vector.tensor_mul(scr[:sz, c, :], scr[:sz, c, :], mag[:sz, c, :])
            nc.vector.tensor_mul(sci[:sz, c, :], sci[:sz, c, :], mag[:sz, c, :])
        tpr = state.tile([128, 4, T], mmdt, tag="tpr", name="tpr")
        tpi = state.tile([128, 4, T], mmdt, tag="tpi", name="tpi")
        epsc = state.tile([128, 1], f32, tag="epsc", name="epsc")
        nc.vector.memset(epsc[...], 1e-30)

        # ---- Griffin-Lim loop -------------------------------------------
        def phase_a_ola():
            """irfft + overlap-add -> (ylo, yhi) each (100, NB)."""
            ylo = ytp.tile([100, NB], mmdt, tag="ylo", name="ylo")
            yhi = ytp.tile([100, NB], mmdt, tag="yhi", name="yhi")
            for m in range(8):
                pm = ps.tile([128, T], f32, tag="ps", name="ps")
                k = 0
                for W, src in ((sb["Wa_r"], scr), (sb["Wa_i"], sci)):
                    for c, (f0, sz) in enumerate(FC):
                        mmul(pm[:100], W[:sz, c, 100 * m:100 * m + 100],
                             src[:sz, c, :], start=(k == 0), stop=(k == 7))
                        k += 1
                tgt, j = (ylo if m % 2 == 0 else yhi), m // 2
                if j == 0:
                    nc.vector.tensor_copy(out=tgt[:, 0:T], in_=pm[:100])
                    nc.vector.memset(tgt[:, T:NB].bitcast(f32), 0.0)
                else:
                    nc.vector.tensor_add(tgt[:, j:j + T], tgt[:, j:j + T],
                                         pm[:100])
            return ylo, yhi

        for it in range(gl_iters):
            ylo, yhi = phase_a_ola()
            yh = (ylo, yhi)
            # edge columns: reads of raw Y first, then writes
            e2lo = work.tile([100, 4], f32, tag="e2", name="e2")
            for i, (h, col, ci) in enumerate(
                    ((0, 2, 0), (1, 2, 1), (0, 512, 2), (1, 512, 3))):
                nc.vector.tensor_mul(e2lo[:, i:i + 1], yh[h][:, col:col + 1],
                                     sb["ecols"][:, ci:ci + 1])
            eps_ = []
            for tcol, ho, srcs in entries:
                ep = ps.tile([128, T], f32, tag="ps", name="ps")
                for si, (hs, blk, mi) in enumerate(srcs):
                    nc.tensor.matmul(
                        ep[:100, 0:1], lhsT=sb["emats"][:, mi, :].bitcast(f32),
                        rhs=yh[hs][:, blk:blk + 1].bitcast(f32),
                        start=(si == 0), stop=(si == len(srcs) - 1))
                eps_.append((tcol, ho, ep))
            for tcol, ho, ep in eps_:
                nc.vector.tensor_copy(out=yh[ho][:, tcol:tcol + 1],
                                      in_=ep[:100, 0:1])
            for i, (h, col, ci) in enumerate(
                    ((0, 2, 0), (1, 2, 1), (0, 512, 2), (1, 512, 3))):
                nc.vector.tensor_copy(out=yh[h][:, col:col + 1],
                                      in_=e2lo[:, i:i + 1])
            # rfft + phase update, per freq chunk
            for c, (f0, sz) in enumerate(FC):
                pr = ps.tile([128, T], f32, tag="ps", name="ps")
                pi = ps.tile([128, T], f32, tag="ps", name="ps")
                for p, W, tp in ((pr, sb["Wc_r"], tpr), (pi, sb["Wc_i"], tpi)):
                    for k in range(8):
                        mmul(p[:sz], W[:, k, f0:f0 + sz],
                             yh[k % 2][:, k // 2:k // 2 + T],
                             start=(k == 0), stop=(k == 7 and it == 0))
                    if it > 0:
                        # fold in -beta * t_prev so psum holds `new` directly
                        mmul(p[:sz], sb["negI"][:sz, :sz], tp[:sz, c, :],
                             start=False, stop=True)
                # tp update: tp = beta*tp + psum  (== rebuilt)
                if it > 0:
                    nc.vector.scalar_tensor_tensor(
                        out=tpr[:sz, c, :], in0=tpr[:sz, c, :], scalar=BETA,
                        in1=pr[:sz], op0=ALU.mult, op1=ALU.add)
                    nc.vector.scalar_tensor_tensor(
                        out=tpi[:sz, c, :], in0=tpi[:sz, c, :], scalar=BETA,
                        in1=pi[:sz], op0=ALU.mult, op1=ALU.add)
                else:
                    nc.scalar.copy(out=tpr[:sz, c, :], in_=pr[:sz])
                    nc.scalar.copy(out=tpi[:sz, c, :], in_=pi[:sz])
                sq = work.tile([128, T], f32, tag="sq", name="sq")
                sqi = work.tile([128, T], f32, tag="sqi", name="sqi")
                nc.scalar.square(out=sq[:sz], in_=pr[:sz])
                nc.scalar.square(out=sqi[:sz], in_=pi[:sz])
                nc.vector.tensor_add(sq[:sz], sq[:sz], sqi[:sz])
                nc.scalar.activation(out=sq[:sz], in_=sq[:sz], func=ACT.Sqrt,
                                     bias=epsc[:sz])
                nc.vector.reciprocal(out=sq[:sz], in_=sq[:sz])
                nc.vector.tensor_mul(sq[:sz], mag[:sz, c, :], sq[:sz])
                nc.vector.tensor_mul(scr[:sz, c, :], pr[:sz], sq[:sz])
                nc.vector.tensor_mul(sci[:sz, c, :], pi[:sz], sq[:sz])

        # ---- final ISTFT + normalize ------------------------------------
        ylo, yhi = phase_a_ola()
        stage = state.tile([100, 2, NQ], f32, tag="stage", name="stage")
        for h, y in ((0, ylo), (1, yhi)):
            nc.vector.tensor_scalar_mul(stage[:, h, :], y[:, 2:2 + NQ],
                                        1.0 / 1.5)
        for h, col, q, ci in ((0, 2, 0, 0), (1, 2, 0, 1),
                              (0, 512, NQ - 1, 2), (1, 512, NQ - 1, 3)):
            y = ylo if h == 0 else yhi
            nc.vector.tensor_mul(stage[:, h, q:q + 1], y[:, col:col + 1],
                                 sb["iwcols"][:, ci:ci + 1])
        rmax = work.tile([100, 2], f32, tag="rmax", name="rmax")
        for h in (0, 1):
            nc.vector.tensor_reduce(out=rmax[:, h:h + 1], in_=stage[:, h, :],
                                    axis=mybir.AxisListType.X, op=ALU.max,
                                    apply_absolute_value=True)
        rm1 = work.tile([100, 1], f32, tag="rm1", name="rm1")
        nc.vector.tensor_max(rm1[...], rmax[:, 0:1], rmax[:, 1:2])
        pt = ps.tile([128, T], f32, tag="ps", name="ps")
        nc.tensor.transpose(pt[:1, :100], rm1[...], ident[:100, :100])
        pk = work.tile([1, 1], f32, tag="pk", name="pk")
        nc.vector.tensor_reduce(out=pk[...], in_=pt[:1, :100],
                                axis=mybir.AxisListType.X, op=ALU.max)
        scl1 = work.tile([1, 1], f32, tag="scl1", name="scl1")
        nc.vector.reciprocal(out=scl1[...], in_=pk[...])
        pb = ps.tile([128, T], f32, tag="ps", name="ps")
        nc.tensor.matmul(pb[:128, 0:1], lhsT=sb["ones1"][:, :],
                         rhs=scl1[...], start=True, stop=True)
        sclc = state.tile([128, 1], f32, tag="sclc", name="sclc")
        nc.vector.tensor_copy(out=sclc[...], in_=pb[:128, 0:1])

        outv = out_d[...].rearrange("(q s) -> q s", s=HOP)
        for c4 in range(4):
            c0 = 128 * c4
            cs = min(128, NQ - c0)
            ost = work.tile([128, HOP], f32, tag="ost", name="ost")
            for h in (0, 1):
                ptt = ps.tile([128, T], f32, tag="ps", name="ps")
                nc.tensor.transpose(ptt[:cs, :100], stage[:, h, c0:c0 + cs],
                                    ident[:100, :100])
                nc.vector.tensor_scalar_mul(ost[:cs, 100 * h:100 * h + 100],
                                            ptt[:cs, :100], sclc[:cs])
            nc.sync.dma_start(out=outv[c0:c0 + cs, :], in_=ost[:cs, :])

    nc.finalize()
    return nc


def _run(x, mm, trace=False, sgd_iters=STFT_ITER, gl_iters=GL_ITER):
    from concourse.bass_utils import run_bass_kernel_spmd

    key = (mm, sgd_iters, gl_iters)
    if key not in _CACHE:
        _CACHE[key] = build_bass(mm, sgd_iters, gl_iters)
    nc = _CACHE[key]

    consts = build_consts(np.float32)
    consts.pop("_entries")
    spec0T, ar, ai = host_inits()

    def chunked(M):   # (401, T) -> (128, 4, T)
        o = np.zeros((128, 4, T), np.float32)
        for c, (f0, sz) in enumerate(FC):
            o[:sz, c, :] = M[f0:f0 + sz, :]
        return o

    in_maps = []
    for b in range(B):
        m = {k: v for k, v in consts.items()}
        m["mel"] = np.ascontiguousarray(x[b], np.float32)
        m["spec0"] = chunked(spec0T[b])
        m["ang_r"] = chunked(ar[b])
        m["ang_i"] = chunked(ai[b])
        in_maps.append(m)

    res = run_bass_kernel_spmd(nc, in_maps, list(range(NCORES)), trace=trace)
    out = np.stack([res.results[b]["out"] for b in range(B)], 0)[:, None, :]
    return out.astype(np.float32), res


def kernel(x):
    mm = os.environ.get("GLV_MMDT", "float32")
    out, _ = _run(np.asarray(x), mm, trace=False)
    return out
